# revision 21
# baseline (speedup 1.0000x reference)
"""Trainium2 Bass kernel for nn_BilinearModule (16,256,64,64 bilinear pooling).

Math (per image):
  y   = relu(bn1(w1 @ x + b1))                       # (32, 4096)
  packed[t] = y[r_t] * y[c_t]  for 528 lower-tri pairs
  out = relu(bn2(w2 @ packed + b2))                  # (256, 4096)

Strategy (pure data parallel over batch, 2 images per core, 8 cores):
  - all matmul operands bf16 (x cast host-side, halves the input DMA);
    fp32 PSUM accumulation and fp32 BN math keep the error ~5e-3.
  - mm1 with M-replicated weights -> psum; fused BN1+ReLU on ACT -> yrep bf16
    (4 identical copies of the 32 channels across 128 partitions).
  - The 528 pair-products: pairs {c, (c+r)%32} for rotations r=0..16.
    r=0 (squares) needs no rotation: GpSimd multiplies yrep*yrep directly.
    r=1..16 live in 4 rotated tiles (4 rotations per tile, one per quadrant)
    produced by 4 K=32 permutation matmuls at tile_position (32i, 0) —
    issued back-to-back with satisfied deps so they run CONCURRENTLY in the
    PE array (row-group tiling, ~1 MM of cost for all 4).
  - Products for rotated tiles: DVE tensor_mul reading PSUM; identity tile
    on GpSimd straight from SBUF.
  - mm2 = 5 K=128 bf16 chunks (4 rotated + identity last) with host-side
    permuted+zero-padded w2; fused BN2+ReLU on ACT -> bf16 store (halves
    the output DMA; host converts to fp32).
  - PE stream per window: mm1(w) | mm2_m0(w-2) | mm2_m1(w-2) | sel(w) —
    12 back-to-back full MMs give BN1(w) 2.2us of slack so the sel group
    never stalls, and the DVE product queue has ~2 windows of slack.
  - x loads ride the ACT hardware DGE ring; output stores + consts ride the
    SP ring, so stores never FIFO-queue behind the 4 MiB of x loads.
  - DMA-free warmup matmuls (memset tile) open the HAM clock-gate during
    the preamble while the first x chunks stream in.
All weights are preprocessed host-side; pair order is folded into w2.
"""

import numpy as np

import concourse.bass as bass
import concourse.mybir as mybir
from concourse import tile
from concourse.bass_utils import run_bass_kernel_spmd

F32 = mybir.dt.float32
BF16 = mybir.dt.bfloat16
AF = mybir.ActivationFunctionType

N_CORES = 8
B, CIN, H, W = 16, 256, 64, 64
NPIX = H * W                     # 4096
IMG_PER_CORE = B // N_CORES      # 2
CMID = 32
COUT = 256
FB = 512                         # pixel window (psum-bank sized)
NWIN = NPIX // FB                # 8 windows per image
NW_TOT = IMG_PER_CORE * NWIN     # 16 windows per core
EPS = 1e-5

# rotation sets per rotated product tile (quadrant q of tile j uses ROTS[j][q]);
# tile 4 is the identity (squares) tile, produced without a matmul.
ROTS = [[1, 2, 3, 4], [5, 6, 7, 8], [9, 10, 11, 12], [13, 14, 15, 16]]

_ctr = [0]


def _split_multi_waits(nc):
    """This container's walrus supports one sync-wait per instruction; split
    extras onto NOP carriers on the same engine."""
    for f in nc.m.functions:
        for blk in f.blocks:
            insts = blk.instructions
            if not any(
                i.sync_info is not None and len(i.sync_info.on_wait) > 1
                for i in insts
            ):
                continue
            new = []
            for inst in insts:
                si = inst.sync_info
                if si is not None and len(si.on_wait) > 1:
                    waits = list(si.on_wait)
                    for wcond in waits[:-1]:
                        _ctr[0] += 1
                        nop = mybir.InstNoOp(name=f"waitnop-{_ctr[0]}", ins=[], outs=[])
                        nop.engine = inst.engine
                        nop.sync_info = mybir.SyncInfo(on_wait=[wcond], on_update=[])
                        new.append(nop)
                    inst.sync_info = mybir.SyncInfo(
                        on_wait=[waits[-1]], on_update=list(si.on_update)
                    )
                new.append(inst)
            blk.instructions = new


def _host_weights(w1, b1, g1, be1, m1, v1, w2, b2, g2, be2, m2, v2):
    """Precompute device weight layouts on the host."""
    # mm1 lhsT, M-replicated: w1t[k, 32q+c] = w1[c, k]
    w1t = np.zeros((CIN, 128), np.float32)
    for q in range(4):
        w1t[:, 32 * q : 32 * q + 32] = w1.T
    inv1 = g1 / np.sqrt(v1 + EPS)
    bn1s = np.tile(inv1, 4).reshape(128, 1).astype(np.float32)
    bn1b = np.tile(b1 * inv1 + be1 - m1 * inv1, 4).reshape(128, 1).astype(np.float32)

    # permutation lhsT for the 4 rotated tiles, one strip per tile:
    # perm[32i + k, 128j + 32q + c] = 1 iff k == (c + ROTS[j][q]) % 32
    # (the sel matmul for tile j reads yrep strip i=j and scatters it into
    #  all 4 output quadrants)
    perm = np.zeros((128, 4 * 128), np.float32)
    for j in range(4):
        i = j
        for q in range(4):
            r = ROTS[j][q]
            for c in range(32):
                k = (c + r) % 32
                perm[32 * i + k, 128 * j + 32 * q + c] = 1.0

    # w2 permuted into the 5x128 product-row order; duplicate slots zeroed.
    off = np.zeros(33, np.int64)
    for d in range(32):
        off[d + 1] = off[d] + (32 - d)
    assert off[32] == 528
    w2p = np.zeros((5 * 128, COUT), np.float32)
    used = np.zeros(528, bool)
    for j in range(4):
        for q in range(4):
            r = ROTS[j][q]
            for c in range(32):
                if r == 16 and c >= 16:
                    continue  # duplicate half of rotation 16
                if c + r < 32:
                    d, b_lo = r, c
                else:
                    d, b_lo = 32 - r, c + r - 32
                t = off[d] + b_lo
                assert not used[t]
                used[t] = True
                w2p[128 * j + 32 * q + c, :] = w2[:, t]
    # identity tile (squares): quadrant 0 rows carry diag-0 weights
    for c in range(32):
        t = off[0] + c
        assert not used[t]
        used[t] = True
        w2p[128 * 4 + c, :] = w2[:, t]
    assert used.all()

    inv2 = g2 / np.sqrt(v2 + EPS)
    bn2s = inv2.reshape(2, 128).T.astype(np.float32).copy()   # [128, 2] col m
    bn2b = (b2 * inv2 + be2 - m2 * inv2).reshape(2, 128).T.astype(np.float32).copy()
    return w1t, bn1s, bn1b, perm, w2p, bn2s, bn2b


def _build_nc():
    nc = bass.Bass()
    # x is host-packed: [img, 128, win, khalf, 512] flattened to [img, 128, 8192]
    # so one contiguous DMA covers both K-halves of a window/quarter.
    x_d = nc.declare_dram_parameter("x", [IMG_PER_CORE, 128, 2 * NPIX], BF16, isOutput=False)
    # bf16 const blobs: [w1a 128 | w1b 128 | perm 512] and [w2p 5*256]
    wp_d = nc.declare_dram_parameter("w1perm", [128, 768], BF16, isOutput=False)
    w2_d = nc.declare_dram_parameter("w2blob", [128, 1280], BF16, isOutput=False)
    # all f32 BN consts in one blob: [bn1s | bn1b | bn2s(2) | bn2b(2)]
    cf_d = nc.declare_dram_parameter("cf32", [128, 6], F32, isOutput=False)
    out_d = nc.declare_dram_parameter("out", [IMG_PER_CORE, COUT, NPIX], BF16, isOutput=True)

    with tile.TileContext(nc) as tc:
        with (
            tc.tile_pool(name="consts", bufs=1) as cpool,
            tc.tile_pool(name="xp", bufs=1) as xpool,
            tc.tile_pool(name="yp", bufs=3) as ypool,
            tc.tile_pool(name="pp", bufs=16) as ppool,
            tc.tile_pool(name="zp", bufs=4) as zpool,
            tc.tile_pool(name="psy", bufs=2, space="PSUM") as psum_y,
            tc.tile_pool(name="pss", bufs=4, space="PSUM") as psum_sel,
            tc.tile_pool(name="psz", bufs=2, space="PSUM") as psum_z,
        ):
            xwin = {}
            xpend = []

            def load_x_win(img, win, eng):
                # one contiguous 256 KB DMA (single DIRECT2D)
                t = xpool.tile([128, 2 * FB], BF16, tag=f"xw{img}{win}")
                eng.dma_start(
                    t[:], x_d[img, :, win * 2 * FB : (win + 1) * 2 * FB]
                )
                xwin[(img, win)] = (t, 0)

            def load_x_quarter(img, h, eng=None):
                # one contiguous 512 KB DMA covering two windows
                t = xpool.tile([128, 4 * FB], BF16, tag=f"xq{img}{h}")
                (eng or nc.scalar).dma_start(
                    t[:], x_d[img, :, h * 4 * FB : (h + 1) * 4 * FB]
                )
                xwin[(img, 2 * h)] = (t, 0)
                xwin[(img, 2 * h + 1)] = (t, 2 * FB)

            # The first two x windows ride the GpSimd SWDGE ring: its
            # descriptor generation starts ~1.2us before the SP sequencer
            # finishes its preamble, so window 0's data lands earliest.
            # Consts ride the SP ring; ACT issues no DMA at all, so BN1 is
            # never queued behind descriptor generation.
            # memset first on the pool queue so the PE warmups start early
            wz = cpool.tile([128, 128], BF16, tag="warmz")
            nc.gpsimd.memset(wz[:], 0.0)
            # SP-ring order: the SDMA engines round-robin all queued
            # transfers, so window 0's first K-half leads, then the weights
            # it needs, then the rest — later-needed blobs trail
            xw0 = xpool.tile([128, 2 * FB], BF16, tag="xw00")
            nc.sync.dma_start(xw0[:, 0:FB], x_d[0, :, 0:FB])
            xwin[(0, 0)] = (xw0, 0)
            wp = cpool.tile([128, 768], BF16, tag="w1perm")
            nc.sync.dma_start(wp[:], wp_d[:])
            nc.sync.dma_start(xw0[:, FB : 2 * FB], x_d[0, :, FB : 2 * FB])
            cf = cpool.tile([128, 6], F32, tag="cf32")
            nc.sync.dma_start(cf[:], cf_d[:])
            load_x_win(0, 1, nc.sync)

            w1a = wp[:, 0:128]
            w1b = wp[:, 128:256]
            perm_sb = wp[:, 256:768]
            bn1s = cf[:, 0:1]
            bn1b = cf[:, 1:2]
            bn2s = cf[:, 2:4]
            bn2b = cf[:, 4:6]

            # Warm the PE clock gate (HAM) while the first x tiles stream in.
            ps_warm = psum_y.tile([128, FB], F32, tag="psy")
            for _ in range(32):
                nc.tensor.matmul(
                    ps_warm[:, 0:128], wz[:], wz[:], start=True, stop=True
                )
            # preload the ACT activation table (one-time ~1.3us) as ACT's
            # first instruction, well before BN1 needs it
            acttab = cpool.tile([128, 1], F32, tag="acttab")
            nc.scalar.activation(acttab[:], wz[:, 0:1], AF.Relu)

            load_x_quarter(0, 1, eng=nc.sync)
            w2p_sb = cpool.tile([128, 1280], BF16, tag="w2blob")
            nc.sync.dma_start(w2p_sb[:], w2_d[:])

            def stage_a1(img, win):
                """mm1 + BN1 for one window."""
                xt, base = xwin[(img, win)]
                xa = xt[:, base : base + FB]
                xb = xt[:, base + FB : base + 2 * FB]
                ps_y = psum_y.tile([128, FB], F32, tag="psy")
                nc.tensor.matmul(ps_y[:], w1a, xa, start=True, stop=False)
                nc.tensor.matmul(ps_y[:], w1b, xb, start=False, stop=True)
                yrep = ypool.tile([128, FB], BF16, tag="yrep")
                nc.scalar.activation(
                    yrep[:], ps_y[:], AF.Relu, bias=bn1b, scale=bn1s
                )
                return yrep

            def stage_sel(yrep):
                """4 concurrent rotation matmuls (one per row-group)."""
                sels = []
                for j in range(4):
                    ps_sel = psum_sel.tile([128, FB], F32, tag="pssel")
                    nc.tensor.matmul(
                        ps_sel[:],
                        perm_sb[32 * j : 32 * j + 32, 128 * j : 128 * (j + 1)],
                        yrep[32 * j : 32 * j + 32, :],
                        start=True,
                        stop=True,
                        tile_position=(32 * j, 0),
                    )
                    sels.append(ps_sel)
                return sels

            def stage_prods(yrep, sels):
                """products: 4 DVE muls from PSUM + identity on GpSimd."""
                prods = []
                for j in range(4):
                    pj = ppool.tile([128, FB], BF16, tag="pj")
                    nc.vector.tensor_mul(pj[:], yrep[:], sels[j][:])
                    prods.append(pj)
                pid = ppool.tile([128, FB], BF16, tag="pid")
                nc.gpsimd.tensor_mul(pid[:], yrep[:], yrep[:])
                prods.append(pid)
                return prods

            def stage_b_m(img, win, prods, m):
                """mm2 m-chunk + BN2 + store for one window."""
                s = slice(win * FB, (win + 1) * FB)
                J_ORDER = (0, 1, 2, 3, 4)  # GpSimd-produced identity chunk last
                ps_z = psum_z.tile([128, FB], F32, tag="psz")
                for idx, j in enumerate(J_ORDER):
                    nc.tensor.matmul(
                        ps_z[:],
                        w2p_sb[:, j * COUT + 128 * m : j * COUT + 128 * m + 128],
                        prods[j][:],
                        start=(idx == 0),
                        stop=(idx == 4),
                    )
                zt = zpool.tile([128, FB], BF16, tag="zt")
                nc.scalar.activation(
                    zt[:], ps_z[:], AF.Relu,
                    bias=bn2b[:, m : m + 1], scale=bn2s[:, m : m + 1],
                )
                nc.sync.dma_start(out_d[img, 128 * m : 128 * m + 128, s], zt[:])

            # software pipeline, PE stream per window:
            #   mm1(w) | mm2_m0(w-2) | mm2_m1(w-2) | sel(w)
            # 12 back-to-back full matmuls give BN1(w) (ACT) ~2.2us of slack
            # before sel(w) needs yrep, and the DVE/GpSimd product queue has
            # ~2 windows of slack before mm2 consumes it.
            wins = [(img, win) for img in range(IMG_PER_CORE) for win in range(NWIN)]
            # remaining x quarters stream in from inside the loop so their
            # descriptor generation never queues ahead of BN1 on the ACT ring
            XLOADS = {0: (0, 2), 1: (0, 3), 2: (1, 0), 3: (1, 1), 4: (1, 2), 5: (1, 3)}
            pipe = []
            for k, (img, win) in enumerate(wins):
                yrep = stage_a1(img, win)
                if len(pipe) == 2:
                    stage_b_m(*pipe[0], 0)
                    stage_b_m(*pipe.pop(0), 1)
                sels = stage_sel(yrep)
                prods = stage_prods(yrep, sels)
                pipe.append((img, win, prods))
                if k in XLOADS:
                    load_x_quarter(*XLOADS[k], eng=nc.sync)
            # drain: products of the final windows are ready well before the
            # earlier windows' mm2 chunks clear the PE. The very last chain
            # (mm2 -> BN2 -> store) is serial-exposed, so split it into
            # half-windows to overlap BN2/store with the second half's mm2.
            for ent in pipe[:-1]:
                stage_b_m(*ent, 0)
                stage_b_m(*ent, 1)
            img, win, prods = pipe[-1]
            stage_b_m(img, win, prods, 0)
            J_ORDER = (0, 1, 2, 3, 4)
            zt = zpool.tile([128, FB], BF16, tag="zt")
            for half in range(2):
                hs = slice(half * (FB // 2), (half + 1) * (FB // 2))
                s = slice(win * FB + half * (FB // 2), win * FB + (half + 1) * (FB // 2))
                # psum_y banks are idle during the drain: using them avoids
                # waiting on the psz bank still being read by BN2(15, m0)
                ps_z = psum_y.tile([128, FB], F32, tag="psy")
                for idx, j in enumerate(J_ORDER):
                    nc.tensor.matmul(
                        ps_z[:, 0 : FB // 2],
                        w2p_sb[:, j * COUT + 128 : j * COUT + 256],
                        prods[j][:, hs],
                        start=(idx == 0),
                        stop=(idx == 4),
                    )
                zh = zt[:, hs]
                nc.scalar.activation(
                    zh, ps_z[:, 0 : FB // 2], AF.Relu,
                    bias=bn2b[:, 1:2], scale=bn2s[:, 1:2],
                )
                nc.sync.dma_start(out_d[img, 128:256, s], zh)

    _split_multi_waits(nc)
    return nc


_cached = {}


def kernel(**inputs):
    x = np.ascontiguousarray(np.asarray(inputs["x"], np.float32))
    args = [
        np.asarray(inputs[k], np.float32)
        for k in ("w1", "b1", "g1", "be1", "m1", "v1", "w2", "b2", "g2", "be2", "m2", "v2")
    ]
    w1t, bn1s, bn1b, perm, w2p, bn2s, bn2b = _host_weights(*args)

    import ml_dtypes
    # bf16 const blobs
    w1perm = np.zeros((128, 768), np.float32)
    w1perm[:, 0:128] = w1t[0:128, :]
    w1perm[:, 128:256] = w1t[128:256, :]
    w1perm[:, 256:768] = perm
    w1perm = w1perm.astype(ml_dtypes.bfloat16)
    w2blob = np.zeros((128, 1280), np.float32)
    for j in range(5):
        w2blob[:, j * COUT : (j + 1) * COUT] = w2p[j * 128 : (j + 1) * 128, :]
    w2blob = w2blob.astype(ml_dtypes.bfloat16)
    # f32 const blob: [bn1s | bn1b | bn2s(2) | bn2b(2)]
    cf32 = np.concatenate([bn1s, bn1b, bn2s, bn2b], axis=1).astype(np.float32)
    assert cf32.shape == (128, 6)
    if "nc" not in _cached:
        _cached["nc"] = _build_nc()
    nc = _cached["nc"]

    # pack x: [B, (a=2) 128ch, 8 win, 512 px] -> [B, 128, win, a, 512] so each
    # window's two K-halves are contiguous per partition in DMA order
    xr = (
        x.reshape(B, 2, 128, NWIN, FB)
        .transpose(0, 2, 3, 1, 4)
        .reshape(B, 128, 2 * NPIX)
        .astype(ml_dtypes.bfloat16)
    )
    shared = {"w1perm": w1perm, "w2blob": w2blob, "cf32": cf32}
    in_maps = [
        {"x": np.ascontiguousarray(xr[c * IMG_PER_CORE : (c + 1) * IMG_PER_CORE]), **shared}
        for c in range(N_CORES)
    ]
    res = run_bass_kernel_spmd(nc, in_maps, core_ids=list(range(N_CORES)))
    kernel.last_results = res
    out = np.concatenate(
        [res.results[c]["out"].astype(np.float32) for c in range(N_CORES)], axis=0
    )
    return out.reshape(B, COUT, H, W)
